# revision 18
# baseline (speedup 1.0000x reference)
"""NetVLAD on 8 Trainium2 NeuronCores — self-contained kernel.

Problem: x [32, 2048, 1024] f32, W [64, 1024] f32, centroids [64, 1024] f32
  -> out [32, 65536] f32  (NetVLAD pooling: per-frame L2 norm, soft-assign
  softmax over 64 clusters, residual aggregation, intra + global L2 norm).

Sharding: data-parallel over batch — 4 samples per core, W/centroids
replicated; no cross-core communication. Per-core program (all bf16 PE work):
  r[m] = 1/||x[m,:]||  (bf16 x; exp/ln chain — ACT Rsqrt is banned)
  z = x @ W^T          (contract D via PE-transposed bf16 x tiles)
  a' = softmax(r*z) * r  (per-tile exp, quarter-level denom reduce+recip)
  agg = a'^T @ x, colsum = a'^T @ ||x||  (= sum_m softmax)  [PE col-packed]
  vlad = agg - colsum*c; per-row L2 normalize; global L2 normalize.

x^T tiles are produced on the PE (identity-matmul transpose, bf16 PSUM out)
and copied back to SBUF on DVE; the xbar transpose DMA of the previous
version is gone, halving DMA-queue occupancy.
"""

import json

import numpy as np

import concourse.bass as bass
import concourse.mybir as mybir
import concourse.tile as tile

F32 = mybir.dt.float32
BF16 = mybir.dt.bfloat16
AF = mybir.ActivationFunctionType
OP = mybir.AluOpType

B = 32
N_CORES = 8
B_PER_CORE = B // N_CORES
M = 2048
D = 1024
K = 64
NQ = 4           # quarters per sample
TQ = 4           # m-tiles per quarter

# Column split of the per-frame sum-of-squares between ACT (first XA cols,
# Square+accumulator) and DVE (rest, tensor_tensor_reduce chained off the
# ACT partial via its initial-value operand).
XA = 640

_PATCHED = False


def _split_waits_json(bir: dict, max_waits: int = 1) -> dict:
    """Split multi-wait sync infos into standalone EventSemaphore waits.

    The walrus build in this image supports a single sync-wait command per
    instruction, while Tile's sem assignment emits several (e.g. the
    kernel-tail Drain waits on every DMAHW lane). Hoisting the extra waits
    into preceding single-wait EventSemaphore instructions on the same
    engine is semantics-preserving for monotonic semaphores.
    """
    ctr = 0
    for f in bir.get("functions", []):
        for blk in f.get("blocks", []):
            insts = blk.get("instructions", [])
            new = []
            for inst in insts:
                si = inst.get("sync_info")
                waits = si.get("on_wait", []) if si else []
                if len(waits) > max_waits:
                    head, keep = waits[:-max_waits], waits[-max_waits:]
                    for w in head:
                        ctr += 1
                        new.append({
                            "debug": inst.get("debug", 0),
                            "engine": inst["engine"],
                            "ins": [],
                            "name": f"{inst['name']}-wsplit{ctr}",
                            "opcode": "EventSemaphore",
                            "outs": [],
                            "sync_info": {"on_update": [], "on_wait": [w]},
                        })
                    si["on_wait"] = keep
                new.append(inst)
            blk["instructions"] = new
    return bir


def _apply_patch():
    global _PATCHED
    if _PATCHED:
        return
    import concourse.bass_utils as bu
    import concourse.bass2jax as b2j
    orig = bu.compile_bir_kernel

    def patched(bir_json, tmpdir, neff_name="file.neff"):
        d = json.loads(bir_json)
        d = _split_waits_json(d, 1)
        return orig(json.dumps(d).encode(), tmpdir, neff_name)

    bu.compile_bir_kernel = patched
    b2j.compile_bir_kernel = patched
    _PATCHED = True


def make_aux_inputs():
    """Constant auxiliary inputs (index/selector matrices, identity)."""
    # Negated so that fs = rv*gb comes out negative and cancels the sign of
    # vladneg = cs*c - agg (computed that way to fuse into one DVE op).
    ind2 = np.zeros((2, 128), dtype=np.float32)
    ind2[0, 0:64] = -1.0
    ind2[1, 64:128] = -1.0
    indK = np.zeros((128, 2), dtype=np.float32)
    indK[0:64, 0] = 1.0
    indK[64:128, 1] = 1.0
    ident = np.eye(128, dtype=np.float32)
    return {"ind2": ind2, "indK": indK, "ident": ident}


def build_nc():
    nc = bass.Bass()
    x = nc.dram_tensor("x", [B_PER_CORE, M, D], F32, kind="ExternalInput")
    W = nc.dram_tensor("W", [K, D], F32, kind="ExternalInput")
    C = nc.dram_tensor("centroids", [K, D], F32, kind="ExternalInput")
    out = nc.dram_tensor("out", [B_PER_CORE, K * D], F32, kind="ExternalOutput")
    ind2_d = nc.dram_tensor("ind2", [2, 128], F32, kind="ExternalInput")
    indK_d = nc.dram_tensor("indK", [128, 2], F32, kind="ExternalInput")
    ident_d = nc.dram_tensor("ident", [128, 128], F32, kind="ExternalInput")

    xr = x[:, :, :].rearrange("s (q t p) d -> s q p t d", q=NQ, t=TQ, p=128)
    outr = out[:, :].rearrange("s (k d) -> s k d", d=D)

    from contextlib import ExitStack
    with tile.TileContext(nc) as tc, ExitStack() as es:
        singles = es.enter_context(tc.tile_pool(name="singles", bufs=1))
        xqpool = es.enter_context(tc.tile_pool(name="xqp", bufs=5))
        xTpool = es.enter_context(tc.tile_pool(name="xTp", bufs=5))
        sqpool = es.enter_context(tc.tile_pool(name="sqp", bufs=3))
        statpool = es.enter_context(tc.tile_pool(name="statp", bufs=4))
        epool = es.enter_context(tc.tile_pool(name="ep", bufs=2))
        apool = es.enter_context(tc.tile_pool(name="apl", bufs=8))
        rspool = es.enter_context(tc.tile_pool(name="rsp", bufs=4))
        tailpool = es.enter_context(tc.tile_pool(name="tailp", bufs=2))
        tppsum = es.enter_context(tc.tile_pool(name="tpps", bufs=3, space="PSUM"))
        zpsum = es.enter_context(tc.tile_pool(name="zps", bufs=2, space="PSUM"))
        aggpsum = es.enter_context(
            tc.tile_pool(name="aggps", bufs=1, space="PSUM"))
        smallps = es.enter_context(
            tc.tile_pool(name="smps", bufs=1, space="PSUM"))

        # Init order matters: ident and W feed the first quarter's transposes
        # and logits, so they go first; the first x quarter is loaded in
        # per-tile pieces so compute starts after ~1/4 of the quarter lands;
        # tail-only constants (cpair, ind2, indK) come last.
        ident = singles.tile([128, 128], BF16)
        nc.gpsimd.dma_start(out=ident, in_=ident_d[:, :])
        Wbf = singles.tile([K, D], BF16)
        nc.gpsimd.dma_start(out=Wbf, in_=W[:, :])
        WT = singles.tile([128, 8, K], BF16)  # WT[q, c, k] = W[k, 128c+q]
        nc.sync.dma_start(out=WT, in_=Wbf, transpose=True)

        st0 = {"s": 0, "q": 0}
        xqq0 = xqpool.tile([128, TQ, D], BF16, tag="xq", name="xq_0_0")
        for i in range(TQ):
            nc.gpsimd.dma_start(out=xqq0[:, i, :], in_=xr[0, 0][:, i, :])
        st0["xqq"] = xqq0

        ind2 = singles.tile([2, 128], F32)
        nc.sync.dma_start(out=ind2, in_=ind2_d[:, :])
        indK = singles.tile([128, 2], F32)
        nc.sync.dma_start(out=indK, in_=indK_d[:, :])
        cpair = singles.tile([128, D], F32)
        nc.gpsimd.dma_start(out=cpair[0:64, :], in_=C[:, :])
        nc.gpsimd.dma_start(out=cpair[64:128, :], in_=C[:, :])

        def emit_load(st):
            s, q = st["s"], st["q"]
            xqq = xqpool.tile([128, TQ, D], BF16, tag="xq", name=f"xq_{s}_{q}")
            nc.gpsimd.dma_start(out=xqq, in_=xr[s, q])
            st["xqq"] = xqq

        def emit_A(st):
            """Norms, PE transposes, copy-back, logits for one quarter."""
            s, q = st["s"], st["q"]
            xqq = st["xqq"]
            ssqa = statpool.tile([128, TQ], F32, tag="ssqa",
                                 name=f"ssqa_{s}_{q}")
            ssqd = statpool.tile([128, TQ], F32, tag="ssqd",
                                 name=f"ssqd_{s}_{q}")
            ssq = statpool.tile([128, TQ], F32, tag="ssq", name=f"ssq_{s}_{q}")
            zq = zpsum.tile([128, TQ, K], F32, tag="zq", name=f"zq_{s}_{q}")
            st["zq"] = zq
            for i in range(TQ):
                sq = sqpool.tile([128, D], BF16, tag="sq", name=f"sq_{s}_{q}_{i}")
                nc.scalar.activation(
                    out=sq[:, 0:XA], in_=xqq[:, i, 0:XA], func=AF.Square,
                    accum_out=ssqa[:, i:i + 1],
                )
                nc.vector.scalar_tensor_tensor(
                    out=sq[:, XA:D], in0=xqq[:, i, XA:D], scalar=1.0,
                    in1=xqq[:, i, XA:D], op0=OP.mult, op1=OP.mult,
                    accum_out=ssqd[:, i:i + 1],
                )
                tp = tppsum.tile([128, 8, 128], BF16, tag="tp",
                                 name=f"tp_{s}_{q}_{i}")
                for c in range(8):
                    nc.tensor.transpose(
                        out=tp[:, c, :], in_=xqq[:, i, 128 * c:128 * (c + 1)],
                        identity=ident,
                    )
                xT = xTpool.tile([128, 8, 128], BF16, tag="xT",
                                 name=f"xT_{s}_{q}_{i}")
                nc.vector.tensor_copy(out=xT[:, 0:4, :], in_=tp[:, 0:4, :])
                nc.vector.tensor_copy(out=xT[:, 4:8, :], in_=tp[:, 4:8, :])
                for c in range(8):
                    nc.tensor.matmul(
                        zq[:, i, :], lhsT=xT[:, c, :], rhs=WT[:, c, :],
                        start=(c == 0), stop=(c == 7),
                    )
            nc.vector.tensor_add(ssq, ssqa, ssqd)
            lnt = statpool.tile([128, TQ], F32, tag="lnt", name=f"lnt_{s}_{q}")
            nc.scalar.activation(out=lnt, in_=ssq, func=AF.Ln)
            r = statpool.tile([128, TQ], F32, tag="r", name=f"r_{s}_{q}")
            nc.scalar.activation(out=r, in_=lnt, func=AF.Exp, scale=-0.5)
            invr = statpool.tile([128, TQ], BF16, tag="invr",
                                 name=f"invr_{s}_{q}")
            nc.scalar.activation(out=invr, in_=lnt, func=AF.Exp, scale=0.5)
            st["r"] = r
            st["invr"] = invr

        def emit_B1(st):
            """Softmax: exps, denominator, assignment weights."""
            s, q = st["s"], st["q"]
            zq, r = st["zq"], st["r"]
            eq = epool.tile([128, TQ, K], BF16, tag="eq", name=f"eq_{s}_{q}")
            for i in range(TQ):
                nc.scalar.activation(
                    out=eq[:, i, :], in_=zq[:, i, :], func=AF.Exp,
                    scale=r[:, i:i + 1],
                )
            sden = statpool.tile([128, TQ], F32, tag="sden",
                                 name=f"sden_{s}_{q}")
            nc.vector.tensor_reduce(
                out=sden, in_=eq, axis=mybir.AxisListType.X, op=OP.add,
            )
            srec = statpool.tile([128, TQ], F32, tag="srec",
                                 name=f"srec_{s}_{q}")
            nc.vector.reciprocal(out=srec, in_=sden)
            st["a"] = []
            for i in range(TQ):
                a = apool.tile([128, K], BF16, tag="a", name=f"a_{s}_{q}_{i}")
                nc.vector.tensor_scalar(
                    out=a, in0=eq[:, i, :], scalar1=srec[:, i:i + 1],
                    scalar2=r[:, i:i + 1], op0=OP.mult, op1=OP.mult,
                )
                st["a"].append(a)

        def emit_Bagg(st, agg, cs):
            """Aggregation matmuls for one quarter."""
            s, q = st["s"], st["q"]
            base = 64 * (s % 2)
            xqq, invr = st["xqq"], st["invr"]
            for i in range(TQ):
                a = st["a"][i]
                st_ = q == 0 and i == 0
                sp_ = q == NQ - 1 and i == TQ - 1
                nc.tensor.matmul(
                    agg[base:base + 64, 0:512], lhsT=a, rhs=xqq[:, i, 0:512],
                    start=st_, stop=sp_,
                )
                nc.tensor.matmul(
                    agg[base:base + 64, 512:1024], lhsT=a,
                    rhs=xqq[:, i, 512:1024], start=st_, stop=sp_,
                )
                nc.tensor.matmul(
                    cs[base:base + 64, 0:1], lhsT=a, rhs=invr[:, i:i + 1],
                    start=st_, stop=sp_,
                )

        def tail_pair(p, agg, cs):
            sa, sb = 2 * p, 2 * p + 1
            cssb = rspool.tile([128, 1], F32, tag="cssb", name=f"cssb_{p}")
            nc.vector.tensor_copy(out=cssb, in_=cs[:, 0:1])
            # vladneg = cs*c - agg (one DVE op; the sign is cancelled by fs<0)
            vlad = tailpool.tile([128, D], F32, tag="vlad", name=f"vlad_{p}")
            nc.vector.scalar_tensor_tensor(
                out=vlad, in0=cpair, scalar=cssb, in1=agg[:, :],
                op0=OP.mult, op1=OP.subtract,
            )
            sq2 = sqpool.tile([128, D], BF16, tag="sq", name=f"sqt_{p}")
            vssq = rspool.tile([128, 1], F32, tag="vssq", name=f"vssq_{p}")
            nc.scalar.activation(out=sq2, in_=vlad, func=AF.Square,
                                 accum_out=vssq)
            lnv = rspool.tile([128, 1], F32, tag="lnv", name=f"lnv_{p}")
            nc.scalar.activation(out=lnv, in_=vssq, func=AF.Ln)
            rv = rspool.tile([128, 1], F32, tag="rv", name=f"rv_{p}")
            nc.scalar.activation(out=rv, in_=lnv, func=AF.Exp, scale=-0.5)
            ssqn = rspool.tile([128, 1], F32, tag="ssqn", name=f"ssqn_{p}")
            nc.vector.tensor_scalar(
                out=ssqn, in0=vssq, scalar1=rv, scalar2=rv,
                op0=OP.mult, op1=OP.mult,
            )
            gsum = smallps.tile([2, 2], F32, tag="smps", name=f"gsum_{p}")
            nc.tensor.matmul(gsum[:, 0:1], lhsT=indK, rhs=ssqn,
                             start=True, stop=True)
            lng = rspool.tile([2, 1], F32, tag="lng", name=f"lng_{p}")
            nc.scalar.activation(out=lng, in_=gsum[:, 0:1], func=AF.Ln)
            ginv = rspool.tile([2, 1], F32, tag="ginv", name=f"ginv_{p}")
            nc.scalar.activation(out=ginv, in_=lng, func=AF.Exp, scale=-0.5)
            gb = smallps.tile([128, 2], F32, tag="smps", name=f"gb_{p}")
            nc.tensor.matmul(gb[:, 0:1], lhsT=ind2, rhs=ginv,
                             start=True, stop=True)
            fs = rspool.tile([128, 1], F32, tag="fs", name=f"fs_{p}")
            nc.vector.tensor_mul(fs, rv, gb[:, 0:1])
            osb = tailpool.tile([128, D], F32, tag="osb", name=f"osb_{p}")
            nc.vector.tensor_scalar_mul(osb, vlad, fs)
            nc.sync.dma_start(out=outr[sa], in_=osb[0:64, :])
            nc.sync.dma_start(out=outr[sb], in_=osb[64:128, :])

        # Software-pipelined: B1 (softmax) runs one quarter behind A, and the
        # agg matmuls two quarters behind, so the PE never waits on the
        # softmax chain and the DVE never blocks the PE's logits feed.
        aggcs = {}

        def get_aggcs(p):
            if p not in aggcs:
                agg = aggpsum.tile([128, D], F32, tag="agg", name=f"agg_{p}")
                cs = smallps.tile([128, 8], F32, tag="smps", name=f"cs_{p}")
                aggcs[p] = (agg, cs)
            return aggcs[p]

        def flush_B(st):
            p = st["s"] // 2
            agg, cs = get_aggcs(p)
            emit_Bagg(st, agg, cs)
            if st["s"] % 2 == 1 and st["q"] == NQ - 1:
                tail_pair(p, agg, cs)

        prev1 = None  # B1 pending
        prev2 = None  # Bagg pending
        for gq in range(B_PER_CORE * NQ):
            s, q = divmod(gq, NQ)
            cur = st0 if gq == 0 else {"s": s, "q": q}
            if gq > 0:
                emit_load(cur)
            if prev1 is not None:
                emit_B1(prev1)
            emit_A(cur)
            if prev2 is not None:
                flush_B(prev2)
            prev2, prev1 = prev1, cur
        emit_B1(prev1)
        flush_B(prev2)
        flush_B(prev1)

    return nc


_NC_CACHE = None


def kernel(**inputs: np.ndarray) -> np.ndarray:
    global _NC_CACHE
    _apply_patch()
    from concourse.bass_utils import run_bass_kernel_spmd

    x = np.ascontiguousarray(np.asarray(inputs["x"], dtype=np.float32))
    W = np.ascontiguousarray(np.asarray(inputs["W"], dtype=np.float32))
    cent = np.ascontiguousarray(
        np.asarray(inputs["centroids"], dtype=np.float32))

    aux = make_aux_inputs()

    if _NC_CACHE is None:
        _NC_CACHE = build_nc()
    nc = _NC_CACHE

    in_maps = [
        dict(
            {
                "x": np.ascontiguousarray(
                    x[B_PER_CORE * c:B_PER_CORE * (c + 1)]),
                "W": W,
                "centroids": cent,
            },
            **aux,
        )
        for c in range(N_CORES)
    ]
    res = run_bass_kernel_spmd(nc, in_maps, core_ids=list(range(N_CORES)))
    return np.concatenate([r["out"] for r in res.results], axis=0)
